# revision 14
# baseline (speedup 1.0000x reference)
"""Data-parallel Trainium2 kernel for PlasticityModelMoE.

Sharding (default g=1, pure batch-parallel): core c owns batch rows
[128c, 128c+128) and computes ALL 8 experts for them in fp16 (10-bit
mantissa, ~2x the error of fp32r but far under the 2e-2 gate; halves every
DMA stream vs f32).  No collectives at all: the 8-way ReduceScatter of the
old expert-parallel layout cost a ~23us CC-stream barrier plus ~15us fixed
cost per collective op, far more than the extra 8MB of expert-weight DMA
this layout pays (g=2 hybrid expert/batch sharding with a pairwise RS is
kept behind MOE_G=2 and measured ~30us slower end-to-end).

Host-side folding: the DynamicConnectivity MLP depends only on neuron_avg
(batch-independent), so cmask = sigmoid(conn)*neuron_mask is computed on the
host and folded into expert_w columns (relu(z*m) == m*relu(z) for m>=0);
device stage 1 is then just gate softmax + z matmuls + relu/gate-scale
accumulation on the DVE.  Columns past the last nonzero mask column are
structurally zero in moe_out, so only h1 columns are computed (and only h1
rows of mem_read_w are loaded).

Engine discipline (the big wins, from perfetto traces):
 - dma_start on an engine BLOCKS that engine when the DGE ring backs up, so
   the scalar(ACT) engine gets only a short DMA queue and its gate-softmax
   Exp is emitted before any bulk dma_start; the sync ring (no compute
   duties) carries most bulk traffic; stage-1 relu/scale/accumulate runs on
   the DVE so PSUM recycling never waits on the ACT queue.
 - Stage 2 (episodic-memory attention + blended learned activation) runs
   with fp16 mem_read_w/memory and is pipelined in two column halves:
   half h+1's read matmuls accumulate on the PE while half h runs its
   activation branches (3 act-table phases per half: ln/exp family,
   sigmoid/tanh, gelu).
"""

import numpy as np

B, D, H, E, M = 1024, 1024, 2048, 8, 2048
NCORES = 8
SELU_SCALE = 1.0507009873554805
SELU_ALPHA = 1.6732632423543772

_CACHED_NC = {}
_LAST_KEY = None
_LAST_IN_MAPS = None


def _build_program(h1, g, has_eb, has_mrb):
    import concourse.bass as bass
    from concourse import bacc, mybir, tile
    from concourse.masks import make_identity

    f32 = mybir.dt.float32
    f32r = mybir.dt.float32r
    f16 = mybir.dt.float16
    EL = E // g          # experts per core
    NB = g               # 128-row batch blocks per core
    NH = h1 // 512       # stage-1 column chunks
    KH = h1 // 128       # K blocks for the attention logits
    AF = mybir.ActivationFunctionType
    ALU = mybir.AluOpType
    AX = mybir.AxisListType

    nc = bacc.Bacc(None, target_bir_lowering=False, debug=False)

    xT_d = nc.dram_tensor("xT", [128, 8, NB * 128], f16, kind="ExternalInput")
    gw_d = nc.dram_tensor("gw", [128, 8, E], f16, kind="ExternalInput")
    ew_d = nc.dram_tensor("ew", [EL, 128, 8, h1], f16, kind="ExternalInput")
    if has_eb:
        eb_d = nc.dram_tensor("eb", [EL, 1, h1], f16, kind="ExternalInput")
    mrw_d = nc.dram_tensor("mrw", [KH, 128, M], f16, kind="ExternalInput")
    if has_mrb:
        mrb_d = nc.dram_tensor("mrb", [1, M], f32, kind="ExternalInput")
    mem_d = nc.dram_tensor("mem", [16, 128, H], f16, kind="ExternalInput")
    cf_d = nc.dram_tensor("coef", [1, 8], f32, kind="ExternalInput")
    out_d = nc.dram_tensor("out", [128, H], f32, kind="ExternalOutput")

    # sync carries the early-critical + most bulk traffic (it has no compute
    # duties so its in-order dma_start queue can block freely); scalar gets a
    # short queue so epilogue ACTs are never blocked behind DMA; gpsimd (slow
    # SW DGE) gets RS traffic, the output, and residual mem tiles.
    dma = nc.default_dma_engine   # SP hwdge ring
    adma = nc.scalar              # ACT hwdge ring
    gdma = nc.gpsimd              # gpsimd SW ring

    with tile.TileContext(nc) as tc:
        with tc.tile_pool(name="consts", bufs=1) as consts, \
             tc.tile_pool(name="dramp", bufs=1, space="DRAM") as dramp, \
             tc.tile_pool(name="mrwp", bufs=KH) as mrwp, \
             tc.tile_pool(name="memp", bufs=16) as memp:

            identity = consts.tile([128, 128], f32, tag="idn")
            make_identity(nc, identity)
            ones_row = consts.tile([1, 128], f32, tag="ones")
            nc.vector.memset(ones_row, 1.0)
            coef_row = consts.tile([1, 8], f32, tag="coef")
            dma.dma_start(coef_row, cf_d[:])
            coeffs_bc = consts.tile([128, 8], f32, tag="cfb")
            moe_r = consts.tile([128, h1], f32, tag="moer")


            if g > 1:
                ys = [dramp.tile([NB * 128, 512], f32, tag=f"y{n}", name=f"y{n}")
                      for n in range(NH)]
                rss = [dramp.tile([128, 512], f32, tag=f"rs{n}", name=f"rs{n}")
                       for n in range(NH)]
                groups = [[2 * k, 2 * k + 1] for k in range(4)]

            # ---------------- stage 1: hybrid-parallel MoE ----------------
            with tc.tile_pool(name="w1", bufs=1) as w1, \
                 tc.tile_pool(name="ewp", bufs=(8 if g > 1 else 4)) as ewp, \
                 tc.tile_pool(name="pb", bufs=1, space="PSUM") as pb:
                xT_sb = w1.tile([128, 8, NB * 128], f16, tag="xT")
                dma.dma_start(xT_sb, xT_d[:])
                gw_sb = w1.tile([128, 8, E], f16, tag="gw")
                adma.dma_start(gw_sb, gw_d[:])
                if has_eb:
                    eb_rows = w1.tile([EL, 1, h1], f16, tag="eb")
                    dma.dma_start(eb_rows, eb_d[:])
                    ones_f16 = w1.tile([1, 128], f16, tag="o16")
                    nc.vector.memset(ones_f16, 1.0)

                # gate softmax FIRST in scalar program order so its Exp
                # is never stuck behind bulk dma_starts on the ACT queue
                gcols = []
                for i in range(NB):
                    bs = slice(i * 128, (i + 1) * 128)
                    gate_ps = pb.tile([128, E], f32, tag="g", bufs=2, name=f"g{i}")
                    for k in range(8):
                        nc.tensor.matmul(gate_ps, xT_sb[:, k, bs], gw_sb[:, k, :],
                                         start=(k == 0), stop=(k == 7))
                    ngm = w1.tile([128, 1], f32, tag="ngm", bufs=2, name=f"ngm{i}")
                    nc.vector.reduce_max(ngm, gate_ps, axis=AX.X, negate=True)
                    eg_t = w1.tile([128, E], f32, tag="eg", bufs=2, name=f"eg{i}")
                    sume = w1.tile([128, 1], f32, tag="se", bufs=2, name=f"se{i}")
                    nc.scalar.activation(eg_t, gate_ps, AF.Exp, bias=ngm,
                                         accum_out=sume)
                    rec = w1.tile([128, 1], f32, tag="rec", bufs=2, name=f"rec{i}")
                    nc.vector.reciprocal(rec, sume)
                    cols = []
                    for j in range(EL):
                        gcol = w1.tile([128, 1], f32, tag=f"gc{i}_{j}",
                                       name=f"gc{i}_{j}")
                        nc.vector.tensor_scalar_mul(gcol, eg_t[:, j:j + 1], rec)
                        cols.append(gcol)
                    gcols.append(cols)

                # expert weight stream: half-expert tiles on two rings
                ew_tiles = []
                for e in range(EL):
                    t0 = ewp.tile([128, 4, h1], f16, tag="ew", name=f"ew{e}h0")
                    dma.dma_start(t0, ew_d[e, :, 0:4, :])
                    t1 = ewp.tile([128, 4, h1], f16, tag="ew", name=f"ew{e}h1")
                    adma.dma_start(t1, ew_d[e, :, 4:8, :])
                    ew_tiles.append((t0, t1))

                # stage-2 weights queued behind the expert stream, split so
                # each ring drains in time for its first consumer
                mrw_tiles = []
                for hk in range(KH):
                    t_ = mrwp.tile([128, M], f16, tag="w", name=f"mrw{hk}")
                    (dma if hk < 4 else adma).dma_start(t_, mrw_d[hk])
                    mrw_tiles.append(t_)
                mem_tiles = []
                for mk in range(16):
                    t_ = memp.tile([128, H], f16, tag="m", name=f"mem{mk}")
                    if g == 1:
                        eng = dma if mk < 5 else (adma if mk < 10 else gdma)
                    else:
                        eng = dma if mk < 6 else (adma if mk < 12 else gdma)
                    eng.dma_start(t_, mem_d[mk])
                    mem_tiles.append(t_)

                # broadcast activation-blend coefficients to 128 partitions
                cf_ps = pb.tile([128, 8], f32, tag="cf")
                nc.tensor.matmul(cf_ps, ones_row, coef_row, start=True, stop=True)
                nc.vector.tensor_copy(coeffs_bc, cf_ps)

                moe_parts = [moe_r] if g == 1 else [
                    w1.tile([128, h1], f32, tag=f"mp{i}", name=f"mp{i}")
                    for i in range(NB)]

                # g>1: chunk-major (all ew resident) so the RS for chunk n
                # fires mid-compute; g==1: expert-major streaming (no RS, and
                # 16 resident half-tiles would not fit SBUF)
                loop_order = ([(n, e) for n in range(NH) for e in range(EL)]
                              if g > 1 else
                              [(n, e) for e in range(EL) for n in range(NH)])
                for n, e in loop_order:
                        sl = slice(n * 512, (n + 1) * 512)
                        t0, t1 = ew_tiles[e]
                        for i in range(NB):
                            bs = slice(i * 128, (i + 1) * 128)
                            z_ps = pb.tile([128, 512], f32, tag="z", bufs=4,
                                           name=f"z{e}_{n}_{i}")
                            for k in range(4):
                                nc.tensor.matmul(z_ps, xT_sb[:, k, bs],
                                                 t0[:, k, sl],
                                                 start=(k == 0), stop=False)
                            for k in range(4):
                                last = (k == 3) and not has_eb
                                nc.tensor.matmul(z_ps, xT_sb[:, 4 + k, bs],
                                                 t1[:, k, sl],
                                                 start=False, stop=last)
                            if has_eb:
                                nc.tensor.matmul(z_ps, ones_f16[0:1, 0:1],
                                                 eb_rows[e, 0:1, sl],
                                                 start=False, stop=True)
                            # relu + gate-scale + accumulate, all on DVE so
                            # the ACT queue never gates PSUM recycling
                            t_ = w1.tile([128, 512], f32, tag="acc", bufs=3,
                                         name=f"a{e}_{n}_{i}")
                            nc.vector.tensor_scalar_max(t_, z_ps, 0.0)
                            if e == 0:
                                nc.vector.tensor_scalar_mul(
                                    moe_parts[i][:, sl], t_, gcols[i][e])
                            else:
                                nc.vector.scalar_tensor_tensor(
                                    moe_parts[i][:, sl], t_, gcols[i][e],
                                    moe_parts[i][:, sl], ALU.mult, ALU.add)

                        # pairwise ReduceScatter as soon as chunk n done
                        if g > 1 and e == EL - 1:
                            for i in range(NB):
                                gdma.dma_start(
                                    ys[n][i * 128:(i + 1) * 128, :],
                                    moe_parts[i][:, sl])
                            nc.gpsimd.collective_compute(
                                "ReduceScatter",
                                bass.mybir.AluOpType.add,
                                replica_groups=groups,
                                ins=[ys[n].opt()],
                                outs=[rss[n].opt()],
                            )
                            gdma.dma_start(moe_r[:, sl], rss[n])

            # ---------------- stage 2: memory read + learned activation ------
            with tc.tile_pool(name="st2", bufs=1) as st2:
                if has_mrb:
                    mrb_row = st2.tile([1, M], f32, tag="mrb")
                    dma.dma_start(mrb_row, mrb_d[:])
                moeT_sb = st2.tile([128, KH * 128], f16, tag="moeT")
                exp_sb = st2.tile([128, M], f32, tag="exp")
                expT_sb = st2.tile([128, 16 * 128], f16, tag="expT")
                s_sb = st2.tile([128, H], f32, tag="s")
                out_sb = st2.tile([128, H], f32, tag="o")
                srec = st2.tile([128, 1], f32, tag="srec")

                with tc.tile_pool(name="pt", bufs=1, space="PSUM") as pt:
                    with tc.tile_pool(name="plg", bufs=1, space="PSUM") as plg:
                        lg = [plg.tile([128, 512], f32, tag="lg", bufs=4,
                                       name=f"lg{n}") for n in range(4)]
                        for ch in range(NH):
                            tp = pt.tile([128, 512], f32, tag="tp", bufs=2,
                                         name=f"tpm{ch}")
                            for j in range(4):
                                hk = ch * 4 + j
                                nc.tensor.transpose(tp[:, j * 128:(j + 1) * 128],
                                                    moe_r[:, hk * 128:(hk + 1) * 128],
                                                    identity)
                            nc.vector.tensor_copy(
                                moeT_sb[:, ch * 512:(ch + 1) * 512], tp)
                            for j in range(4):
                                hk = ch * 4 + j
                                for n in range(4):
                                    nc.tensor.matmul(
                                        lg[n],
                                        moeT_sb[:, hk * 128:(hk + 1) * 128],
                                        mrw_tiles[hk][:, n * 512:(n + 1) * 512],
                                        start=(hk == 0),
                                        stop=(hk == KH - 1) and not has_mrb)
                        if has_mrb:
                            for n in range(4):
                                nc.tensor.matmul(lg[n], ones_row[0:1, 0:1],
                                                 mrb_row[0:1, n * 512:(n + 1) * 512],
                                                 start=False, stop=True)

                        nmx = []
                        for n in range(4):
                            t_ = st2.tile([128, 1], f32, tag=f"nmx{n}", name=f"nmx{n}")
                            nc.vector.reduce_max(t_, lg[n], axis=AX.X, negate=True)
                            nmx.append(t_)
                        t01 = st2.tile([128, 1], f32, tag="t01")
                        nc.vector.tensor_scalar_min(t01, nmx[0], nmx[1])
                        t23 = st2.tile([128, 1], f32, tag="t23")
                        nc.vector.tensor_scalar_min(t23, nmx[2], nmx[3])
                        ngm2 = st2.tile([128, 1], f32, tag="ngm2")
                        nc.vector.tensor_scalar_min(ngm2, t01, t23)
                        ses = []
                        for n in range(4):
                            se_ = st2.tile([128, 1], f32, tag=f"ses{n}", name=f"ses{n}")
                            nc.scalar.activation(exp_sb[:, n * 512:(n + 1) * 512],
                                                 lg[n], AF.Exp, bias=ngm2,
                                                 accum_out=se_)
                            ses.append(se_)
                        s01 = st2.tile([128, 1], f32, tag="s01")
                        nc.vector.tensor_tensor(s01, ses[0], ses[1], ALU.add)
                        s23 = st2.tile([128, 1], f32, tag="s23")
                        nc.vector.tensor_tensor(s23, ses[2], ses[3], ALU.add)
                        stot = st2.tile([128, 1], f32, tag="stot")
                        nc.vector.tensor_tensor(stot, s01, s23, ALU.add)
                        nc.vector.reciprocal(srec, stot)

                    for t in range(4):
                        tp = pt.tile([128, 512], f32, tag="tp", bufs=2, name=f"tpe{t}")
                        for j in range(4):
                            mk = t * 4 + j
                            nc.tensor.transpose(tp[:, j * 128:(j + 1) * 128],
                                                exp_sb[:, mk * 128:(mk + 1) * 128],
                                                identity)
                        nc.vector.tensor_copy(expT_sb[:, t * 512:(t + 1) * 512],
                                              tp)

                # read matmul + blended activation, pipelined in column
                # halves: while half h runs its activation branches, half h+1
                # accumulates its read matmuls on the PE.
                # Mish is synthesized without softplus/ln:
                #   tanh(softplus(s)) == 1 - 2/((1+e^s)^2 + 1)  (s clamped at
                #   20, where the expression saturates to 1 in f32).
                with tc.tile_pool(name="prd", bufs=1, space="PSUM") as prd, \
                     tc.tile_pool(name="pac", bufs=1, space="PSUM") as pac, \
                     tc.tile_pool(name="brp", bufs=1) as brp:
                    HH = H // 2
                    n_groups = 7

                    for h in range(2):
                        hs = slice(h * HH, (h + 1) * HH)
                        rd = [prd.tile([128, 512], f32, tag="rd", bufs=4,
                                       name=f"rd{h}_{n}") for n in range(2)]
                        for mk in range(16):
                            for n in range(2):
                                nc.tensor.matmul(
                                    rd[n],
                                    expT_sb[:, mk * 128:(mk + 1) * 128],
                                    mem_tiles[mk][:, h * HH + n * 512:
                                                  h * HH + (n + 1) * 512],
                                    start=(mk == 0), stop=(mk == 15))
                        # s = moe + read_vec/sum (deferred normalization);
                        # columns >= h1 have moe == 0 by mask structure
                        for n in range(2):
                            sl = slice(h * HH + n * 512, h * HH + (n + 1) * 512)
                            if h * HH + n * 512 < h1:
                                nc.vector.scalar_tensor_tensor(
                                    s_sb[:, sl], rd[n], srec, moe_r[:, sl],
                                    ALU.mult, ALU.add)
                            else:
                                nc.vector.tensor_scalar_mul(s_sb[:, sl], rd[n],
                                                            srec)

                        s_h = s_sb[:, hs]
                        acc = [pac.tile([128, 512], f32, tag="acc", bufs=4,
                                        name=f"acc{h}_{n}") for n in range(2)]
                        gi = [0]

                        def acc_branch(br_tile, ci):
                            diag = brp.tile([128, 128], f32r, tag="d", bufs=2,
                                            name=f"d{h}_{gi[0]}")
                            nc.vector.tensor_scalar_mul(diag, identity,
                                                        coeffs_bc[:, ci:ci + 1])
                            for n in range(2):
                                nc.tensor.matmul(acc[n], diag,
                                                 br_tile[:, n * 512:(n + 1) * 512],
                                                 start=(gi[0] == 0),
                                                 stop=(gi[0] == n_groups - 1))
                            gi[0] += 1

                        # --- nl_exp table phase ---
                        relu_br = brp.tile([128, HH], f32r, tag="relu",
                                           bufs=2, name=f"rl{h}")
                        nc.scalar.activation(relu_br, s_h, AF.Relu)
                        acc_branch(relu_br, 5)
                        # exp(min(s,0)); the -1 of expm1 is folded into the
                        # final subtraction of c_em below
                        mn = brp.tile([128, HH], f32, tag="sc1", bufs=2,
                                      name=f"mn{h}")
                        nc.vector.tensor_scalar_min(mn, s_h, 0.0)
                        em_br = brp.tile([128, HH], f32r, tag="b", bufs=2,
                                         name=f"em{h}")
                        nc.scalar.activation(em_br, mn, AF.Exp)
                        acc_branch(em_br, 6)
                        # mish = s * tanh(relu(s) + ln(1 + exp(-|s|)));
                        # abs/exp/ln/relu share one activation table
                        abs_s = brp.tile([128, HH], f32, tag="sc2", bufs=2,
                                         name=f"ab{h}")
                        nc.scalar.activation(abs_s, s_h, AF.Abs)
                        enab = brp.tile([128, HH], f32, tag="sc1", bufs=2,
                                        name=f"en{h}")
                        nc.scalar.activation(enab, abs_s, AF.Exp, scale=-1.0)
                        ep1 = brp.tile([128, HH], f32, tag="sc2", bufs=2,
                                       name=f"e1{h}")
                        nc.vector.tensor_scalar_add(ep1, enab, 1.0)
                        ln1p = brp.tile([128, HH], f32, tag="sc1", bufs=2,
                                        name=f"ln{h}")
                        nc.scalar.activation(ln1p, ep1, AF.Ln)
                        sp = brp.tile([128, HH], f32, tag="sc2", bufs=2,
                                      name=f"sp{h}")
                        nc.vector.tensor_tensor(sp, ln1p, relu_br.bitcast(f32),
                                                ALU.add)
                        mt = brp.tile([128, HH], f32, tag="sc1", bufs=2,
                                      name=f"mt{h}")
                        nc.scalar.activation(mt, sp, AF.Tanh)
                        mish_br = brp.tile([128, HH], f32r, tag="b", bufs=2,
                                           name=f"mi{h}")
                        nc.vector.tensor_tensor(mish_br, mt, s_h, ALU.mult)
                        acc_branch(mish_br, 4)
                        # --- sigmoid table phase ---
                        sg_br = brp.tile([128, HH], f32r, tag="b", bufs=2,
                                         name=f"sg{h}")
                        nc.scalar.activation(sg_br, s_h, AF.Sigmoid)
                        acc_branch(sg_br, 0)
                        th_br = brp.tile([128, HH], f32r, tag="b", bufs=2,
                                         name=f"th{h}")
                        nc.scalar.activation(th_br, s_h, AF.Tanh)
                        acc_branch(th_br, 1)
                        # silu = s * sigmoid(s), on the vector engine
                        sl_br = brp.tile([128, HH], f32r, tag="b", bufs=2,
                                         name=f"sl{h}")
                        nc.vector.tensor_tensor(sl_br, sg_br.bitcast(f32), s_h,
                                                ALU.mult)
                        acc_branch(sl_br, 2)
                        # --- gelu table phase ---
                        gl_br = brp.tile([128, HH], f32r, tag="b", bufs=2,
                                         name=f"gl{h}")
                        nc.scalar.activation(gl_br, s_h, AF.Gelu)
                        acc_branch(gl_br, 3)
                        assert gi[0] == n_groups
                        for n in range(2):
                            sl = slice(h * HH + n * 512, h * HH + (n + 1) * 512)
                            nc.vector.tensor_scalar_sub(out_sb[:, sl], acc[n],
                                                        coeffs_bc[:, 6:7])
                        dma.dma_start(out_d[:, hs], out_sb[:, hs])
    nc.finalize()
    return nc


def _get_nc(key=None):
    if key is None:
        key = _LAST_KEY
    if key not in _CACHED_NC:
        _CACHED_NC[key] = _build_program(*key)
    return _CACHED_NC[key]


def _r12(a):
    """Round fp32 to the fp32r grid (11 explicit mantissa bits, RNE)."""
    u = np.ascontiguousarray(a).view(np.uint32)
    u = (u + np.uint32(0x7FF) + ((u >> np.uint32(12)) & np.uint32(1))) \
        & np.uint32(0xFFFFF000)
    return u.view(np.float32)


def kernel(**inputs):
    import os
    from concourse.bass_utils import run_bass_kernel_spmd

    f = lambda a: np.ascontiguousarray(np.asarray(a, dtype=np.float32))
    x = f(inputs["x"])
    gate_w = f(inputs["gate_w"])
    expert_w = f(inputs["expert_w"])
    expert_b = f(inputs["expert_b"])
    conn_w1 = f(inputs["conn_w1"])
    conn_b1 = f(inputs["conn_b1"])
    conn_w2 = f(inputs["conn_w2"])
    conn_b2 = f(inputs["conn_b2"])
    neuron_avg = f(inputs["neuron_avg"])
    neuron_mask = f(inputs["neuron_mask"])
    mem_read_w = f(inputs["mem_read_w"])
    mem_read_b = f(inputs["mem_read_b"])
    memory = f(inputs["memory"])
    act_w = f(inputs["act_w"]).reshape(-1)

    g = int(os.environ.get("MOE_G", "1"))
    EL = E // g
    NB = g

    # host prep: softmax blend weights -> 7 branch coefficients
    p = np.exp(act_w - act_w.max())
    p = p / p.sum()
    coef = np.array([[p[0], p[2], p[4], p[5], p[7],
                      p[1] + p[3] + p[6] * SELU_SCALE,
                      p[1] + p[6] * SELU_SCALE * SELU_ALPHA, 0.0]], np.float32)

    # host conn MLP (batch-independent) -> cmask folded into expert weights
    h1v = np.einsum('eh,ehk->ek', neuron_avg, conn_w1) + conn_b1
    h1v = np.maximum(h1v, 0.0, dtype=np.float32)
    cl = np.einsum('ek,ekh->eh', h1v, conn_w2) + conn_b2
    conn = (1.0 / (1.0 + np.exp(-cl))).astype(np.float32)
    cmask = conn * neuron_mask                                   # [E, H]

    # stage-1 live width: columns past the last nonzero mask column are
    # structurally zero in moe_out, so the program skips them entirely
    nz = np.nonzero(neuron_mask.any(axis=0))[0]
    h1 = int(nz[-1]) + 1 if nz.size else 512
    h1 = min(H, max(512, -(-h1 // 512) * 512))

    wp = (expert_w[:, :, :h1] * cmask[:, None, :h1]).astype(np.float16)
    bp = (expert_b[:, :h1] * cmask[:, :h1]).astype(np.float16)
    has_eb = bool(np.any(bp))
    has_mrb = bool(np.any(mem_read_b))

    xT = np.ascontiguousarray(x.T).astype(np.float16)            # [D, B]
    xT_blk = xT.reshape(8, 128, B).transpose(1, 0, 2)            # [128, 8, B]
    mrw_bf = mem_read_w[:h1].reshape(h1 // 128, 128, M).astype(np.float16)
    mem_bf = memory.reshape(16, 128, H).astype(np.float16)
    mrb = np.ascontiguousarray(mem_read_b.reshape(1, M))

    in_maps = []
    for c in range(NCORES):
        if g > 1:
            bg, eg = c >> 1, c & 1
        else:
            bg, eg = c, 0
        gwr = np.roll(gate_w, -eg * EL, axis=1).astype(np.float16)
        ew_c = wp[eg * EL:(eg + 1) * EL]         # [EL, D, h1]
        m = {
            "xT": np.ascontiguousarray(
                xT_blk[:, :, bg * NB * 128:(bg + 1) * NB * 128]),
            "gw": np.ascontiguousarray(gwr.reshape(8, 128, E).transpose(1, 0, 2)),
            "ew": np.ascontiguousarray(
                ew_c.reshape(EL, 8, 128, h1).transpose(0, 2, 1, 3)),
            "mrw": mrw_bf,
            "mem": mem_bf,
            "coef": coef,
        }
        if has_eb:
            m["eb"] = np.ascontiguousarray(
                bp[eg * EL:(eg + 1) * EL].reshape(EL, 1, h1))
        if has_mrb:
            m["mrb"] = mrb
        in_maps.append(m)

    key = (h1, g, has_eb, has_mrb)
    global _LAST_IN_MAPS, _LAST_KEY
    _LAST_IN_MAPS = in_maps
    _LAST_KEY = key
    nc = _get_nc(key)
    results = run_bass_kernel_spmd(nc, in_maps, list(range(NCORES))).results
    out = np.concatenate(
        [np.asarray(results[c]["out"], dtype=np.float32) for c in range(NCORES)],
        axis=0)
    return out


# revision 15
# speedup vs baseline: 1.0228x; 1.0228x over previous
"""Data-parallel Trainium2 kernel for PlasticityModelMoE.

Sharding (default g=1, pure batch-parallel): core c owns batch rows
[128c, 128c+128) and computes ALL 8 experts for them in fp16 (10-bit
mantissa, ~2x the error of fp32r but far under the 2e-2 gate; halves every
DMA stream vs f32).  No collectives at all: the 8-way ReduceScatter of the
old expert-parallel layout cost a ~23us CC-stream barrier plus ~15us fixed
cost per collective op, far more than the extra 8MB of expert-weight DMA
this layout pays (g=2 hybrid expert/batch sharding with a pairwise RS is
kept behind MOE_G=2 and measured ~30us slower end-to-end).

Host-side folding: the DynamicConnectivity MLP depends only on neuron_avg
(batch-independent), so cmask = sigmoid(conn)*neuron_mask is computed on the
host and folded into expert_w columns (relu(z*m) == m*relu(z) for m>=0);
device stage 1 is then just gate softmax + z matmuls + relu/gate-scale
accumulation on the DVE.  Columns past the last nonzero mask column are
structurally zero in moe_out, so only h1 columns are computed (and only h1
rows of mem_read_w are loaded).

Engine discipline (the big wins, from perfetto traces):
 - dma_start on an engine BLOCKS that engine when the DGE ring backs up, so
   the scalar(ACT) engine gets only a short DMA queue and its gate-softmax
   Exp is emitted before any bulk dma_start; the sync ring (no compute
   duties) carries most bulk traffic; stage-1 relu/scale/accumulate runs on
   the DVE so PSUM recycling never waits on the ACT queue.
 - Stage 2 (episodic-memory attention + blended learned activation) runs
   with fp16 mem_read_w/memory and is pipelined in two column halves:
   half h+1's read matmuls accumulate on the PE while half h runs its
   activation branches (3 act-table phases per half: ln/exp family,
   sigmoid/tanh, gelu).
"""

import numpy as np

B, D, H, E, M = 1024, 1024, 2048, 8, 2048
NCORES = 8
SELU_SCALE = 1.0507009873554805
SELU_ALPHA = 1.6732632423543772

_CACHED_NC = {}
_LAST_KEY = None
_LAST_IN_MAPS = None


def _build_program(h1, g, has_eb, has_mrb):
    import concourse.bass as bass
    from concourse import bacc, mybir, tile
    from concourse.masks import make_identity

    f32 = mybir.dt.float32
    f32r = mybir.dt.float32r
    f16 = mybir.dt.float16
    EL = E // g          # experts per core
    NB = g               # 128-row batch blocks per core
    NH = h1 // 512       # stage-1 column chunks
    KH = h1 // 128       # K blocks for the attention logits
    AF = mybir.ActivationFunctionType
    ALU = mybir.AluOpType
    AX = mybir.AxisListType

    nc = bacc.Bacc(None, target_bir_lowering=False, debug=False)

    xT_d = nc.dram_tensor("xT", [128, 8, NB * 128], f16, kind="ExternalInput")
    gw_d = nc.dram_tensor("gw", [128, 8, E], f16, kind="ExternalInput")
    ew_d = nc.dram_tensor("ew", [EL, 128, 8, h1], f16, kind="ExternalInput")
    if has_eb:
        eb_d = nc.dram_tensor("eb", [EL, 1, h1], f16, kind="ExternalInput")
    mrw_d = nc.dram_tensor("mrw", [KH, 128, M], f16, kind="ExternalInput")
    if has_mrb:
        mrb_d = nc.dram_tensor("mrb", [1, M], f32, kind="ExternalInput")
    mem_d = nc.dram_tensor("mem", [16, 128, H], f16, kind="ExternalInput")
    cf_d = nc.dram_tensor("coef", [1, 8], f32, kind="ExternalInput")
    out_d = nc.dram_tensor("out", [128, H], f32, kind="ExternalOutput")

    # sync carries the early-critical + most bulk traffic (it has no compute
    # duties so its in-order dma_start queue can block freely); scalar gets a
    # short queue so epilogue ACTs are never blocked behind DMA; gpsimd (slow
    # SW DGE) gets RS traffic, the output, and residual mem tiles.
    dma = nc.default_dma_engine   # SP hwdge ring
    adma = nc.scalar              # ACT hwdge ring
    gdma = nc.gpsimd              # gpsimd SW ring

    with tile.TileContext(nc) as tc:
        with tc.tile_pool(name="consts", bufs=1) as consts, \
             tc.tile_pool(name="dramp", bufs=1, space="DRAM") as dramp, \
             tc.tile_pool(name="mrwp", bufs=KH) as mrwp, \
             tc.tile_pool(name="memp", bufs=16) as memp:

            identity = consts.tile([128, 128], f32, tag="idn")
            make_identity(nc, identity)
            ones_row = consts.tile([1, 128], f32, tag="ones")
            nc.vector.memset(ones_row, 1.0)
            coef_row = consts.tile([1, 8], f32, tag="coef")
            dma.dma_start(coef_row, cf_d[:])
            coeffs_bc = consts.tile([128, 8], f32, tag="cfb")
            moe_r = consts.tile([128, h1], f32, tag="moer")


            if g > 1:
                ys = [dramp.tile([NB * 128, 512], f32, tag=f"y{n}", name=f"y{n}")
                      for n in range(NH)]
                rss = [dramp.tile([128, 512], f32, tag=f"rs{n}", name=f"rs{n}")
                       for n in range(NH)]
                groups = [[2 * k, 2 * k + 1] for k in range(4)]

            # ---------------- stage 1: hybrid-parallel MoE ----------------
            with tc.tile_pool(name="w1", bufs=1) as w1, \
                 tc.tile_pool(name="ewp", bufs=(8 if g > 1 else 4)) as ewp, \
                 tc.tile_pool(name="pb", bufs=1, space="PSUM") as pb:
                xT_sb = w1.tile([128, 8, NB * 128], f16, tag="xT")
                dma.dma_start(xT_sb, xT_d[:])
                gw_sb = w1.tile([128, 8, E], f16, tag="gw")
                adma.dma_start(gw_sb, gw_d[:])
                if has_eb:
                    eb_rows = w1.tile([EL, 1, h1], f16, tag="eb")
                    dma.dma_start(eb_rows, eb_d[:])
                    ones_f16 = w1.tile([1, 128], f16, tag="o16")
                    nc.vector.memset(ones_f16, 1.0)

                # gate softmax FIRST in scalar program order so its Exp
                # is never stuck behind bulk dma_starts on the ACT queue
                gcols = []
                for i in range(NB):
                    bs = slice(i * 128, (i + 1) * 128)
                    gate_ps = pb.tile([128, E], f32, tag="g", bufs=2, name=f"g{i}")
                    for k in range(8):
                        nc.tensor.matmul(gate_ps, xT_sb[:, k, bs], gw_sb[:, k, :],
                                         start=(k == 0), stop=(k == 7))
                    ngm = w1.tile([128, 1], f32, tag="ngm", bufs=2, name=f"ngm{i}")
                    nc.vector.reduce_max(ngm, gate_ps, axis=AX.X, negate=True)
                    eg_t = w1.tile([128, E], f32, tag="eg", bufs=2, name=f"eg{i}")
                    sume = w1.tile([128, 1], f32, tag="se", bufs=2, name=f"se{i}")
                    nc.scalar.activation(eg_t, gate_ps, AF.Exp, bias=ngm,
                                         accum_out=sume)
                    rec = w1.tile([128, 1], f32, tag="rec", bufs=2, name=f"rec{i}")
                    nc.vector.reciprocal(rec, sume)
                    cols = []
                    for j in range(EL):
                        gcol = w1.tile([128, 1], f32, tag=f"gc{i}_{j}",
                                       name=f"gc{i}_{j}")
                        nc.vector.tensor_scalar_mul(gcol, eg_t[:, j:j + 1], rec)
                        cols.append(gcol)
                    gcols.append(cols)

                # expert weight stream: half-expert tiles on two rings
                ew_tiles = []
                for e in range(EL):
                    t0 = ewp.tile([128, 4, h1], f16, tag="ew", name=f"ew{e}h0")
                    dma.dma_start(t0, ew_d[e, :, 0:4, :])
                    t1 = ewp.tile([128, 4, h1], f16, tag="ew", name=f"ew{e}h1")
                    adma.dma_start(t1, ew_d[e, :, 4:8, :])
                    ew_tiles.append((t0, t1))

                # stage-2 weights queued behind the expert stream, split so
                # each ring drains in time for its first consumer
                mrw_tiles = []
                for hk in range(KH):
                    t_ = mrwp.tile([128, M], f16, tag="w", name=f"mrw{hk}")
                    (dma if hk < 4 else adma).dma_start(t_, mrw_d[hk])
                    mrw_tiles.append(t_)
                mem_tiles = []
                for mk in range(16):
                    t_ = memp.tile([128, H], f16, tag="m", name=f"mem{mk}")
                    if g == 1:
                        eng = dma if mk < 5 else (adma if mk < 10 else gdma)
                    else:
                        eng = dma if mk < 6 else (adma if mk < 12 else gdma)
                    eng.dma_start(t_, mem_d[mk])
                    mem_tiles.append(t_)

                # broadcast activation-blend coefficients to 128 partitions
                cf_ps = pb.tile([128, 8], f32, tag="cf")
                nc.tensor.matmul(cf_ps, ones_row, coef_row, start=True, stop=True)
                nc.vector.tensor_copy(coeffs_bc, cf_ps)

                moe_parts = [moe_r] if g == 1 else [
                    w1.tile([128, h1], f32, tag=f"mp{i}", name=f"mp{i}")
                    for i in range(NB)]

                # g>1: chunk-major (all ew resident) so the RS for chunk n
                # fires mid-compute; g==1: expert-major streaming (no RS, and
                # 16 resident half-tiles would not fit SBUF)
                loop_order = ([(n, e) for n in range(NH) for e in range(EL)]
                              if g > 1 else
                              [(n, e) for e in range(EL) for n in range(NH)])
                for n, e in loop_order:
                        sl = slice(n * 512, (n + 1) * 512)
                        t0, t1 = ew_tiles[e]
                        for i in range(NB):
                            bs = slice(i * 128, (i + 1) * 128)
                            z_ps = pb.tile([128, 512], f32, tag="z", bufs=4,
                                           name=f"z{e}_{n}_{i}")
                            for k in range(4):
                                nc.tensor.matmul(z_ps, xT_sb[:, k, bs],
                                                 t0[:, k, sl],
                                                 start=(k == 0), stop=False)
                            for k in range(4):
                                last = (k == 3) and not has_eb
                                nc.tensor.matmul(z_ps, xT_sb[:, 4 + k, bs],
                                                 t1[:, k, sl],
                                                 start=False, stop=last)
                            if has_eb:
                                nc.tensor.matmul(z_ps, ones_f16[0:1, 0:1],
                                                 eb_rows[e, 0:1, sl],
                                                 start=False, stop=True)
                            # relu + gate-scale + accumulate, all on DVE so
                            # the ACT queue never gates PSUM recycling
                            t_ = w1.tile([128, 512], f32, tag="acc", bufs=3,
                                         name=f"a{e}_{n}_{i}")
                            nc.vector.tensor_scalar_max(t_, z_ps, 0.0)
                            if e == 0:
                                nc.vector.tensor_scalar_mul(
                                    moe_parts[i][:, sl], t_, gcols[i][e])
                            else:
                                nc.vector.scalar_tensor_tensor(
                                    moe_parts[i][:, sl], t_, gcols[i][e],
                                    moe_parts[i][:, sl], ALU.mult, ALU.add)

                        # pairwise ReduceScatter as soon as chunk n done
                        if g > 1 and e == EL - 1:
                            for i in range(NB):
                                gdma.dma_start(
                                    ys[n][i * 128:(i + 1) * 128, :],
                                    moe_parts[i][:, sl])
                            nc.gpsimd.collective_compute(
                                "ReduceScatter",
                                bass.mybir.AluOpType.add,
                                replica_groups=groups,
                                ins=[ys[n].opt()],
                                outs=[rss[n].opt()],
                            )
                            gdma.dma_start(moe_r[:, sl], rss[n])

            # ---------------- stage 2: memory read + learned activation ------
            with tc.tile_pool(name="st2", bufs=1) as st2:
                if has_mrb:
                    mrb_row = st2.tile([1, M], f32, tag="mrb")
                    dma.dma_start(mrb_row, mrb_d[:])
                moeT_sb = st2.tile([128, KH * 128], f16, tag="moeT")
                exp_sb = st2.tile([128, M], f32, tag="exp")
                expT_sb = st2.tile([128, 16 * 128], f16, tag="expT")
                s_sb = st2.tile([128, H], f32, tag="s")
                out_sb = st2.tile([128, H], f32, tag="o")
                srec = st2.tile([128, 1], f32, tag="srec")

                with tc.tile_pool(name="pt", bufs=1, space="PSUM") as pt:
                    with tc.tile_pool(name="plg", bufs=1, space="PSUM") as plg:
                        lg = [plg.tile([128, 512], f32, tag="lg", bufs=4,
                                       name=f"lg{n}") for n in range(4)]
                        for ch in range(NH):
                            tp = pt.tile([128, 512], f32, tag="tp", bufs=2,
                                         name=f"tpm{ch}")
                            for j in range(4):
                                hk = ch * 4 + j
                                nc.tensor.transpose(tp[:, j * 128:(j + 1) * 128],
                                                    moe_r[:, hk * 128:(hk + 1) * 128],
                                                    identity)
                            nc.vector.tensor_copy(
                                moeT_sb[:, ch * 512:(ch + 1) * 512], tp)
                            for j in range(4):
                                hk = ch * 4 + j
                                for n in range(4):
                                    nc.tensor.matmul(
                                        lg[n],
                                        moeT_sb[:, hk * 128:(hk + 1) * 128],
                                        mrw_tiles[hk][:, n * 512:(n + 1) * 512],
                                        start=(hk == 0),
                                        stop=(hk == KH - 1) and not has_mrb)
                        if has_mrb:
                            for n in range(4):
                                nc.tensor.matmul(lg[n], ones_row[0:1, 0:1],
                                                 mrb_row[0:1, n * 512:(n + 1) * 512],
                                                 start=False, stop=True)

                        nmx = []
                        for n in range(4):
                            t_ = st2.tile([128, 1], f32, tag=f"nmx{n}", name=f"nmx{n}")
                            nc.vector.reduce_max(t_, lg[n], axis=AX.X, negate=True)
                            nmx.append(t_)
                        t01 = st2.tile([128, 1], f32, tag="t01")
                        nc.vector.tensor_scalar_min(t01, nmx[0], nmx[1])
                        t23 = st2.tile([128, 1], f32, tag="t23")
                        nc.vector.tensor_scalar_min(t23, nmx[2], nmx[3])
                        ngm2 = st2.tile([128, 1], f32, tag="ngm2")
                        nc.vector.tensor_scalar_min(ngm2, t01, t23)
                        ses = []
                        for n in range(4):
                            se_ = st2.tile([128, 1], f32, tag=f"ses{n}", name=f"ses{n}")
                            nc.scalar.activation(exp_sb[:, n * 512:(n + 1) * 512],
                                                 lg[n], AF.Exp, bias=ngm2,
                                                 accum_out=se_)
                            ses.append(se_)
                        s01 = st2.tile([128, 1], f32, tag="s01")
                        nc.vector.tensor_tensor(s01, ses[0], ses[1], ALU.add)
                        s23 = st2.tile([128, 1], f32, tag="s23")
                        nc.vector.tensor_tensor(s23, ses[2], ses[3], ALU.add)
                        stot = st2.tile([128, 1], f32, tag="stot")
                        nc.vector.tensor_tensor(stot, s01, s23, ALU.add)
                        nc.vector.reciprocal(srec, stot)

                    for t in range(4):
                        tp = pt.tile([128, 512], f32, tag="tp", bufs=2, name=f"tpe{t}")
                        for j in range(4):
                            mk = t * 4 + j
                            nc.tensor.transpose(tp[:, j * 128:(j + 1) * 128],
                                                exp_sb[:, mk * 128:(mk + 1) * 128],
                                                identity)
                        nc.vector.tensor_copy(expT_sb[:, t * 512:(t + 1) * 512],
                                              tp)

                # read matmul + blended activation, pipelined in column
                # halves: while half h runs its activation branches, half h+1
                # accumulates its read matmuls on the PE.
                # Mish is synthesized without softplus/ln:
                #   tanh(softplus(s)) == 1 - 2/((1+e^s)^2 + 1)  (s clamped at
                #   20, where the expression saturates to 1 in f32).
                with tc.tile_pool(name="prd", bufs=1, space="PSUM") as prd, \
                     tc.tile_pool(name="pac", bufs=1, space="PSUM") as pac, \
                     tc.tile_pool(name="brp", bufs=1) as brp:
                    HH = H // 2
                    n_groups = 7

                    for h in range(2):
                        hs = slice(h * HH, (h + 1) * HH)
                        rd = [prd.tile([128, 512], f32, tag="rd", bufs=4,
                                       name=f"rd{h}_{n}") for n in range(2)]
                        for mk in range(16):
                            for n in range(2):
                                nc.tensor.matmul(
                                    rd[n],
                                    expT_sb[:, mk * 128:(mk + 1) * 128],
                                    mem_tiles[mk][:, h * HH + n * 512:
                                                  h * HH + (n + 1) * 512],
                                    start=(mk == 0), stop=(mk == 15))
                        # s = moe + read_vec/sum (deferred normalization);
                        # columns >= h1 have moe == 0 by mask structure
                        for n in range(2):
                            sl = slice(h * HH + n * 512, h * HH + (n + 1) * 512)
                            if h * HH + n * 512 < h1:
                                nc.vector.scalar_tensor_tensor(
                                    s_sb[:, sl], rd[n], srec, moe_r[:, sl],
                                    ALU.mult, ALU.add)
                            else:
                                nc.vector.tensor_scalar_mul(s_sb[:, sl], rd[n],
                                                            srec)

                        s_h = s_sb[:, hs]
                        acc = [pac.tile([128, 512], f32, tag="acc", bufs=4,
                                        name=f"acc{h}_{n}") for n in range(2)]
                        gi = [0]

                        def acc_branch(br_tile, ci):
                            diag = brp.tile([128, 128], f32r, tag="d", bufs=2,
                                            name=f"d{h}_{gi[0]}")
                            nc.vector.tensor_scalar_mul(diag, identity,
                                                        coeffs_bc[:, ci:ci + 1])
                            for n in range(2):
                                nc.tensor.matmul(acc[n], diag,
                                                 br_tile[:, n * 512:(n + 1) * 512],
                                                 start=(gi[0] == 0),
                                                 stop=(gi[0] == n_groups - 1))
                            gi[0] += 1

                        # --- nl_exp table phase ---
                        relu_br = brp.tile([128, HH], f32r, tag="relu",
                                           bufs=2, name=f"rl{h}")
                        nc.scalar.activation(relu_br, s_h, AF.Relu)
                        acc_branch(relu_br, 5)
                        # exp(min(s,0)); the -1 of expm1 is folded into the
                        # final subtraction of c_em below
                        mn = brp.tile([128, HH], f32, tag="sc1", bufs=2,
                                      name=f"mn{h}")
                        nc.vector.tensor_scalar_min(mn, s_h, 0.0)
                        em_br = brp.tile([128, HH], f32r, tag="b", bufs=2,
                                         name=f"em{h}")
                        nc.scalar.activation(em_br, mn, AF.Exp)
                        acc_branch(em_br, 6)
                        # mish = s * tanh(softplus(s)) computed without
                        # ln/tanh ACT passes (ACT is the tail bottleneck):
                        # tanh(softplus(s)) == 1 - 2/((1+e^s)^2 + 1), exact
                        # for s <= 20 and saturated to 1 in f32 above it; the
                        # DVE reciprocal is slow but overlaps the ACT stream.
                        mn20 = brp.tile([128, HH], f32, tag="sc2", bufs=2,
                                        name=f"m20{h}")
                        nc.vector.tensor_scalar_min(mn20, s_h, 20.0)
                        v_br = brp.tile([128, HH], f32, tag="sc1", bufs=2,
                                        name=f"v{h}")
                        nc.scalar.activation(v_br, mn20, AF.Exp)
                        vp1 = brp.tile([128, HH], f32, tag="sc2", bufs=2,
                                       name=f"vp{h}")
                        nc.vector.tensor_scalar_add(vp1, v_br, 1.0)
                        w_t = brp.tile([128, HH], f32, tag="sc1", bufs=2,
                                       name=f"w{h}")
                        nc.vector.tensor_tensor(w_t, vp1, vp1, ALU.mult)
                        wp1 = brp.tile([128, HH], f32, tag="sc2", bufs=2,
                                       name=f"wp{h}")
                        nc.vector.tensor_scalar_add(wp1, w_t, 1.0)
                        r_t = brp.tile([128, HH], f32, tag="sc1", bufs=2,
                                       name=f"r{h}")
                        nc.vector.reciprocal(r_t, wp1)
                        tsp = brp.tile([128, HH], f32, tag="sc2", bufs=2,
                                       name=f"t{h}")
                        nc.vector.tensor_scalar(tsp, r_t, -2.0, 1.0,
                                                ALU.mult, ALU.add)
                        mish_br = brp.tile([128, HH], f32r, tag="b", bufs=2,
                                           name=f"mi{h}")
                        nc.vector.tensor_tensor(mish_br, tsp, s_h, ALU.mult)
                        acc_branch(mish_br, 4)
                        # --- sigmoid table phase ---
                        sg_br = brp.tile([128, HH], f32r, tag="b", bufs=2,
                                         name=f"sg{h}")
                        nc.scalar.activation(sg_br, s_h, AF.Sigmoid)
                        acc_branch(sg_br, 0)
                        th_br = brp.tile([128, HH], f32r, tag="b", bufs=2,
                                         name=f"th{h}")
                        nc.scalar.activation(th_br, s_h, AF.Tanh)
                        acc_branch(th_br, 1)
                        # silu = s * sigmoid(s), on the vector engine
                        sl_br = brp.tile([128, HH], f32r, tag="b", bufs=2,
                                         name=f"sl{h}")
                        nc.vector.tensor_tensor(sl_br, sg_br.bitcast(f32), s_h,
                                                ALU.mult)
                        acc_branch(sl_br, 2)
                        # --- gelu table phase ---
                        gl_br = brp.tile([128, HH], f32r, tag="b", bufs=2,
                                         name=f"gl{h}")
                        nc.scalar.activation(gl_br, s_h, AF.Gelu)
                        acc_branch(gl_br, 3)
                        assert gi[0] == n_groups
                        for n in range(2):
                            sl = slice(h * HH + n * 512, h * HH + (n + 1) * 512)
                            nc.vector.tensor_scalar_sub(out_sb[:, sl], acc[n],
                                                        coeffs_bc[:, 6:7])
                        dma.dma_start(out_d[:, hs], out_sb[:, hs])
    nc.finalize()
    return nc


def _get_nc(key=None):
    if key is None:
        key = _LAST_KEY
    if key not in _CACHED_NC:
        _CACHED_NC[key] = _build_program(*key)
    return _CACHED_NC[key]


def _r12(a):
    """Round fp32 to the fp32r grid (11 explicit mantissa bits, RNE)."""
    u = np.ascontiguousarray(a).view(np.uint32)
    u = (u + np.uint32(0x7FF) + ((u >> np.uint32(12)) & np.uint32(1))) \
        & np.uint32(0xFFFFF000)
    return u.view(np.float32)


def kernel(**inputs):
    import os
    from concourse.bass_utils import run_bass_kernel_spmd

    f = lambda a: np.ascontiguousarray(np.asarray(a, dtype=np.float32))
    x = f(inputs["x"])
    gate_w = f(inputs["gate_w"])
    expert_w = f(inputs["expert_w"])
    expert_b = f(inputs["expert_b"])
    conn_w1 = f(inputs["conn_w1"])
    conn_b1 = f(inputs["conn_b1"])
    conn_w2 = f(inputs["conn_w2"])
    conn_b2 = f(inputs["conn_b2"])
    neuron_avg = f(inputs["neuron_avg"])
    neuron_mask = f(inputs["neuron_mask"])
    mem_read_w = f(inputs["mem_read_w"])
    mem_read_b = f(inputs["mem_read_b"])
    memory = f(inputs["memory"])
    act_w = f(inputs["act_w"]).reshape(-1)

    g = int(os.environ.get("MOE_G", "1"))
    EL = E // g
    NB = g

    # host prep: softmax blend weights -> 7 branch coefficients
    p = np.exp(act_w - act_w.max())
    p = p / p.sum()
    coef = np.array([[p[0], p[2], p[4], p[5], p[7],
                      p[1] + p[3] + p[6] * SELU_SCALE,
                      p[1] + p[6] * SELU_SCALE * SELU_ALPHA, 0.0]], np.float32)

    # host conn MLP (batch-independent) -> cmask folded into expert weights
    h1v = np.einsum('eh,ehk->ek', neuron_avg, conn_w1) + conn_b1
    h1v = np.maximum(h1v, 0.0, dtype=np.float32)
    cl = np.einsum('ek,ekh->eh', h1v, conn_w2) + conn_b2
    conn = (1.0 / (1.0 + np.exp(-cl))).astype(np.float32)
    cmask = conn * neuron_mask                                   # [E, H]

    # stage-1 live width: columns past the last nonzero mask column are
    # structurally zero in moe_out, so the program skips them entirely
    nz = np.nonzero(neuron_mask.any(axis=0))[0]
    h1 = int(nz[-1]) + 1 if nz.size else 512
    h1 = min(H, max(512, -(-h1 // 512) * 512))

    wp = (expert_w[:, :, :h1] * cmask[:, None, :h1]).astype(np.float16)
    bp = (expert_b[:, :h1] * cmask[:, :h1]).astype(np.float16)
    has_eb = bool(np.any(bp))
    has_mrb = bool(np.any(mem_read_b))

    xT = np.ascontiguousarray(x.T).astype(np.float16)            # [D, B]
    xT_blk = xT.reshape(8, 128, B).transpose(1, 0, 2)            # [128, 8, B]
    mrw_bf = mem_read_w[:h1].reshape(h1 // 128, 128, M).astype(np.float16)
    mem_bf = memory.reshape(16, 128, H).astype(np.float16)
    mrb = np.ascontiguousarray(mem_read_b.reshape(1, M))

    in_maps = []
    for c in range(NCORES):
        if g > 1:
            bg, eg = c >> 1, c & 1
        else:
            bg, eg = c, 0
        gwr = np.roll(gate_w, -eg * EL, axis=1).astype(np.float16)
        ew_c = wp[eg * EL:(eg + 1) * EL]         # [EL, D, h1]
        m = {
            "xT": np.ascontiguousarray(
                xT_blk[:, :, bg * NB * 128:(bg + 1) * NB * 128]),
            "gw": np.ascontiguousarray(gwr.reshape(8, 128, E).transpose(1, 0, 2)),
            "ew": np.ascontiguousarray(
                ew_c.reshape(EL, 8, 128, h1).transpose(0, 2, 1, 3)),
            "mrw": mrw_bf,
            "mem": mem_bf,
            "coef": coef,
        }
        if has_eb:
            m["eb"] = np.ascontiguousarray(
                bp[eg * EL:(eg + 1) * EL].reshape(EL, 1, h1))
        if has_mrb:
            m["mrb"] = mrb
        in_maps.append(m)

    key = (h1, g, has_eb, has_mrb)
    global _LAST_IN_MAPS, _LAST_KEY
    _LAST_IN_MAPS = in_maps
    _LAST_KEY = key
    nc = _get_nc(key)
    results = run_bass_kernel_spmd(nc, in_maps, list(range(NCORES))).results
    out = np.concatenate(
        [np.asarray(results[c]["out"], dtype=np.float32) for c in range(NCORES)],
        axis=0)
    return out


# revision 16
# speedup vs baseline: 1.0758x; 1.0519x over previous
"""Data-parallel Trainium2 kernel for PlasticityModelMoE.

Sharding (default g=1, pure batch-parallel): core c owns batch rows
[128c, 128c+128) and computes ALL 8 experts for them in fp16 (10-bit
mantissa, ~2x the error of fp32r but far under the 2e-2 gate; halves every
DMA stream vs f32).  No collectives at all: the 8-way ReduceScatter of the
old expert-parallel layout cost a ~23us CC-stream barrier plus ~15us fixed
cost per collective op, far more than the extra 8MB of expert-weight DMA
this layout pays (g=2 hybrid expert/batch sharding with a pairwise RS is
kept behind MOE_G=2 and measured ~30us slower end-to-end).

Host-side folding: the DynamicConnectivity MLP depends only on neuron_avg
(batch-independent), so cmask = sigmoid(conn)*neuron_mask is computed on the
host and folded into expert_w columns (relu(z*m) == m*relu(z) for m>=0);
device stage 1 is then just gate softmax + z matmuls + relu/gate-scale
accumulation on the DVE.  Columns past the last nonzero mask column are
structurally zero in moe_out, so only h1 columns are computed (and only h1
rows of mem_read_w are loaded).

Engine discipline (the big wins, from perfetto traces):
 - dma_start on an engine BLOCKS that engine when the DGE ring backs up, so
   the scalar(ACT) engine gets only a short DMA queue and its gate-softmax
   Exp is emitted before any bulk dma_start; the sync ring (no compute
   duties) carries most bulk traffic; stage-1 relu/scale/accumulate runs on
   the DVE so PSUM recycling never waits on the ACT queue.
 - Stage 2 (episodic-memory attention + blended learned activation) runs
   with fp16 mem_read_w/memory and is pipelined in two column halves:
   half h+1's read matmuls accumulate on the PE while half h runs its
   activation branches (3 act-table phases per half: ln/exp family,
   sigmoid/tanh, gelu).
"""

import numpy as np

B, D, H, E, M = 1024, 1024, 2048, 8, 2048
NCORES = 8
SELU_SCALE = 1.0507009873554805
SELU_ALPHA = 1.6732632423543772

_CACHED_NC = {}
_LAST_KEY = None
_LAST_IN_MAPS = None


def _build_program(h1, g, has_eb, has_mrb):
    import concourse.bass as bass
    from concourse import bacc, mybir, tile
    from concourse.masks import make_identity

    f32 = mybir.dt.float32
    f32r = mybir.dt.float32r
    f16 = mybir.dt.float16
    EL = E // g          # experts per core
    NB = g               # 128-row batch blocks per core
    NH = h1 // 512       # stage-1 column chunks
    KH = h1 // 128       # K blocks for the attention logits
    AF = mybir.ActivationFunctionType
    ALU = mybir.AluOpType
    AX = mybir.AxisListType

    nc = bacc.Bacc(None, target_bir_lowering=False, debug=False)

    xT_d = nc.dram_tensor("xT", [128, 8, NB * 128], f16, kind="ExternalInput")
    gw_d = nc.dram_tensor("gw", [128, 8, E], f16, kind="ExternalInput")
    ew_d = nc.dram_tensor("ew", [EL, 128, 8, h1], f16, kind="ExternalInput")
    if has_eb:
        eb_d = nc.dram_tensor("eb", [EL, 1, h1], f16, kind="ExternalInput")
    mrw_d = nc.dram_tensor("mrw", [KH, 128, M], f16, kind="ExternalInput")
    if has_mrb:
        mrb_d = nc.dram_tensor("mrb", [1, M], f32, kind="ExternalInput")
    mem_d = nc.dram_tensor("mem", [16, 128, H], f16, kind="ExternalInput")
    cf_d = nc.dram_tensor("coef", [1, 8], f32, kind="ExternalInput")
    out_d = nc.dram_tensor("out", [128, H], f32, kind="ExternalOutput")

    # sync carries the early-critical + most bulk traffic (it has no compute
    # duties so its in-order dma_start queue can block freely); scalar gets a
    # short queue so epilogue ACTs are never blocked behind DMA; gpsimd (slow
    # SW DGE) gets RS traffic, the output, and residual mem tiles.
    dma = nc.default_dma_engine   # SP hwdge ring
    adma = nc.scalar              # ACT hwdge ring
    gdma = nc.gpsimd              # gpsimd SW ring

    with tile.TileContext(nc) as tc:
        with tc.tile_pool(name="consts", bufs=1) as consts, \
             tc.tile_pool(name="dramp", bufs=1, space="DRAM") as dramp, \
             tc.tile_pool(name="mrwp", bufs=KH) as mrwp, \
             tc.tile_pool(name="memp", bufs=16) as memp:

            identity = consts.tile([128, 128], f32, tag="idn")
            make_identity(nc, identity)
            ones_row = consts.tile([1, 128], f32, tag="ones")
            nc.vector.memset(ones_row, 1.0)
            coef_row = consts.tile([1, 8], f32, tag="coef")
            dma.dma_start(coef_row, cf_d[:])
            coeffs_bc = consts.tile([128, 8], f32, tag="cfb")
            moe_r = consts.tile([128, h1], f32, tag="moer")


            if g > 1:
                ys = [dramp.tile([NB * 128, 512], f32, tag=f"y{n}", name=f"y{n}")
                      for n in range(NH)]
                rss = [dramp.tile([128, 512], f32, tag=f"rs{n}", name=f"rs{n}")
                       for n in range(NH)]
                groups = [[2 * k, 2 * k + 1] for k in range(4)]

            # ---------------- stage 1: hybrid-parallel MoE ----------------
            with tc.tile_pool(name="w1", bufs=1) as w1, \
                 tc.tile_pool(name="ewp", bufs=(8 if g > 1 else 4)) as ewp, \
                 tc.tile_pool(name="pb", bufs=1, space="PSUM") as pb:
                xT_sb = w1.tile([128, 8, NB * 128], f16, tag="xT")
                dma.dma_start(xT_sb, xT_d[:])
                gw_sb = w1.tile([128, 8, E], f16, tag="gw")
                adma.dma_start(gw_sb, gw_d[:])
                if has_eb:
                    eb_rows = w1.tile([EL, 1, h1], f16, tag="eb")
                    dma.dma_start(eb_rows, eb_d[:])
                    ones_f16 = w1.tile([1, 128], f16, tag="o16")
                    nc.vector.memset(ones_f16, 1.0)

                # gate softmax FIRST in scalar program order so its Exp
                # is never stuck behind bulk dma_starts on the ACT queue
                gcols = []
                for i in range(NB):
                    bs = slice(i * 128, (i + 1) * 128)
                    gate_ps = pb.tile([128, E], f32, tag="g", bufs=2, name=f"g{i}")
                    for k in range(8):
                        nc.tensor.matmul(gate_ps, xT_sb[:, k, bs], gw_sb[:, k, :],
                                         start=(k == 0), stop=(k == 7))
                    eg_t = w1.tile([128, E], f32, tag="eg", bufs=2, name=f"eg{i}")
                    sume = w1.tile([128, 1], f32, tag="se", bufs=2, name=f"se{i}")
                    nc.scalar.activation(eg_t, gate_ps, AF.Exp,
                                         accum_out=sume)
                    rec = w1.tile([128, 1], f32, tag="rec", bufs=2, name=f"rec{i}")
                    nc.vector.reciprocal(rec, sume)
                    cols = []
                    for j in range(EL):
                        gcol = w1.tile([128, 1], f32, tag=f"gc{i}_{j}",
                                       name=f"gc{i}_{j}")
                        nc.vector.tensor_scalar_mul(gcol, eg_t[:, j:j + 1], rec)
                        cols.append(gcol)
                    gcols.append(cols)

                # expert weight stream: half-expert tiles on two rings
                ew_tiles = []
                for e in range(EL):
                    t0 = ewp.tile([128, 4, h1], f16, tag="ew", name=f"ew{e}h0")
                    dma.dma_start(t0, ew_d[e, :, 0:4, :])
                    t1 = ewp.tile([128, 4, h1], f16, tag="ew", name=f"ew{e}h1")
                    adma.dma_start(t1, ew_d[e, :, 4:8, :])
                    ew_tiles.append((t0, t1))

                # stage-2 weights queued behind the expert stream, split so
                # each ring drains in time for its first consumer
                mrw_tiles = []
                for hk in range(KH):
                    t_ = mrwp.tile([128, M], f16, tag="w", name=f"mrw{hk}")
                    (dma if hk < 4 else adma).dma_start(t_, mrw_d[hk])
                    mrw_tiles.append(t_)
                mem_tiles = []
                for mk in range(16):
                    t_ = memp.tile([128, H], f16, tag="m", name=f"mem{mk}")
                    if g == 1:
                        eng = dma if mk < 5 else (adma if mk < 10 else gdma)
                    else:
                        eng = dma if mk < 6 else (adma if mk < 12 else gdma)
                    eng.dma_start(t_, mem_d[mk])
                    mem_tiles.append(t_)

                # broadcast activation-blend coefficients to 128 partitions
                cf_ps = pb.tile([128, 8], f32, tag="cf")
                nc.tensor.matmul(cf_ps, ones_row, coef_row, start=True, stop=True)
                nc.vector.tensor_copy(coeffs_bc, cf_ps)

                moe_parts = [moe_r] if g == 1 else [
                    w1.tile([128, h1], f32, tag=f"mp{i}", name=f"mp{i}")
                    for i in range(NB)]

                # g>1: chunk-major (all ew resident) so the RS for chunk n
                # fires mid-compute; g==1: expert-major streaming (no RS, and
                # 16 resident half-tiles would not fit SBUF)
                loop_order = ([(n, e) for n in range(NH) for e in range(EL)]
                              if g > 1 else
                              [(n, e) for e in range(EL) for n in range(NH)])
                for n, e in loop_order:
                        sl = slice(n * 512, (n + 1) * 512)
                        t0, t1 = ew_tiles[e]
                        for i in range(NB):
                            bs = slice(i * 128, (i + 1) * 128)
                            z_ps = pb.tile([128, 512], f32, tag="z", bufs=4,
                                           name=f"z{e}_{n}_{i}")
                            for k in range(4):
                                nc.tensor.matmul(z_ps, xT_sb[:, k, bs],
                                                 t0[:, k, sl],
                                                 start=(k == 0), stop=False)
                            for k in range(4):
                                last = (k == 3) and not has_eb
                                nc.tensor.matmul(z_ps, xT_sb[:, 4 + k, bs],
                                                 t1[:, k, sl],
                                                 start=False, stop=last)
                            if has_eb:
                                nc.tensor.matmul(z_ps, ones_f16[0:1, 0:1],
                                                 eb_rows[e, 0:1, sl],
                                                 start=False, stop=True)
                            # relu + gate-scale + accumulate, all on DVE so
                            # the ACT queue never gates PSUM recycling
                            t_ = w1.tile([128, 512], f32, tag="acc", bufs=3,
                                         name=f"a{e}_{n}_{i}")
                            nc.vector.tensor_scalar_max(t_, z_ps, 0.0)
                            if e == 0:
                                nc.vector.tensor_scalar_mul(
                                    moe_parts[i][:, sl], t_, gcols[i][e])
                            else:
                                nc.vector.scalar_tensor_tensor(
                                    moe_parts[i][:, sl], t_, gcols[i][e],
                                    moe_parts[i][:, sl], ALU.mult, ALU.add)

                        # pairwise ReduceScatter as soon as chunk n done
                        if g > 1 and e == EL - 1:
                            for i in range(NB):
                                gdma.dma_start(
                                    ys[n][i * 128:(i + 1) * 128, :],
                                    moe_parts[i][:, sl])
                            nc.gpsimd.collective_compute(
                                "ReduceScatter",
                                bass.mybir.AluOpType.add,
                                replica_groups=groups,
                                ins=[ys[n].opt()],
                                outs=[rss[n].opt()],
                            )
                            gdma.dma_start(moe_r[:, sl], rss[n])

            # ---------------- stage 2: memory read + learned activation ------
            with tc.tile_pool(name="st2", bufs=1) as st2:
                if has_mrb:
                    mrb_row = st2.tile([1, M], f32, tag="mrb")
                    dma.dma_start(mrb_row, mrb_d[:])
                moeT_sb = st2.tile([128, KH * 128], f16, tag="moeT")
                exp_sb = st2.tile([128, M], f32, tag="exp")
                expT_sb = st2.tile([128, 16 * 128], f16, tag="expT")
                s_sb = st2.tile([128, H], f32, tag="s")
                out_sb = st2.tile([128, H], f32, tag="o")
                srec = st2.tile([128, 1], f32, tag="srec")

                with tc.tile_pool(name="pt", bufs=1, space="PSUM") as pt:
                    with tc.tile_pool(name="plg", bufs=1, space="PSUM") as plg:
                        lg = [plg.tile([128, 512], f32, tag="lg", bufs=4,
                                       name=f"lg{n}") for n in range(4)]
                        for ch in range(NH):
                            tp = pt.tile([128, 512], f32, tag="tp", bufs=2,
                                         name=f"tpm{ch}")
                            for j in range(4):
                                hk = ch * 4 + j
                                nc.tensor.transpose(tp[:, j * 128:(j + 1) * 128],
                                                    moe_r[:, hk * 128:(hk + 1) * 128],
                                                    identity)
                            nc.vector.tensor_copy(
                                moeT_sb[:, ch * 512:(ch + 1) * 512], tp)
                            for j in range(4):
                                hk = ch * 4 + j
                                for n in range(4):
                                    nc.tensor.matmul(
                                        lg[n],
                                        moeT_sb[:, hk * 128:(hk + 1) * 128],
                                        mrw_tiles[hk][:, n * 512:(n + 1) * 512],
                                        start=(hk == 0),
                                        stop=(hk == KH - 1) and not has_mrb)
                        if has_mrb:
                            for n in range(4):
                                nc.tensor.matmul(lg[n], ones_row[0:1, 0:1],
                                                 mrb_row[0:1, n * 512:(n + 1) * 512],
                                                 start=False, stop=True)

                        # logits here are O(1), so exp needs no max
                        # subtraction (normalization is already deferred via
                        # srec) — each chunk's exp + transpose can fire as
                        # soon as its PSUM bank stops accumulating
                        ses = []
                        for n in range(4):
                            se_ = st2.tile([128, 1], f32, tag=f"ses{n}", name=f"ses{n}")
                            nc.scalar.activation(exp_sb[:, n * 512:(n + 1) * 512],
                                                 lg[n], AF.Exp,
                                                 accum_out=se_)
                            ses.append(se_)
                        s01 = st2.tile([128, 1], f32, tag="s01")
                        nc.vector.tensor_tensor(s01, ses[0], ses[1], ALU.add)
                        s23 = st2.tile([128, 1], f32, tag="s23")
                        nc.vector.tensor_tensor(s23, ses[2], ses[3], ALU.add)
                        stot = st2.tile([128, 1], f32, tag="stot")
                        nc.vector.tensor_tensor(stot, s01, s23, ALU.add)
                        nc.vector.reciprocal(srec, stot)

                    for t in range(4):
                        tp = pt.tile([128, 512], f32, tag="tp", bufs=2, name=f"tpe{t}")
                        for j in range(4):
                            mk = t * 4 + j
                            nc.tensor.transpose(tp[:, j * 128:(j + 1) * 128],
                                                exp_sb[:, mk * 128:(mk + 1) * 128],
                                                identity)
                        nc.vector.tensor_copy(expT_sb[:, t * 512:(t + 1) * 512],
                                              tp)

                # read matmul + blended activation, pipelined in column
                # halves: while half h runs its activation branches, half h+1
                # accumulates its read matmuls on the PE.
                # Mish is synthesized without softplus/ln:
                #   tanh(softplus(s)) == 1 - 2/((1+e^s)^2 + 1)  (s clamped at
                #   20, where the expression saturates to 1 in f32).
                with tc.tile_pool(name="prd", bufs=1, space="PSUM") as prd, \
                     tc.tile_pool(name="pac", bufs=1, space="PSUM") as pac, \
                     tc.tile_pool(name="brp", bufs=1) as brp:
                    HH = H // 2
                    n_groups = 7

                    for h in range(2):
                        hs = slice(h * HH, (h + 1) * HH)
                        rd = [prd.tile([128, 512], f32, tag="rd", bufs=4,
                                       name=f"rd{h}_{n}") for n in range(2)]
                        for mk in range(16):
                            for n in range(2):
                                nc.tensor.matmul(
                                    rd[n],
                                    expT_sb[:, mk * 128:(mk + 1) * 128],
                                    mem_tiles[mk][:, h * HH + n * 512:
                                                  h * HH + (n + 1) * 512],
                                    start=(mk == 0), stop=(mk == 15))
                        # s = moe + read_vec/sum (deferred normalization);
                        # columns >= h1 have moe == 0 by mask structure
                        for n in range(2):
                            sl = slice(h * HH + n * 512, h * HH + (n + 1) * 512)
                            if h * HH + n * 512 < h1:
                                nc.vector.scalar_tensor_tensor(
                                    s_sb[:, sl], rd[n], srec, moe_r[:, sl],
                                    ALU.mult, ALU.add)
                            else:
                                nc.vector.tensor_scalar_mul(s_sb[:, sl], rd[n],
                                                            srec)

                        s_h = s_sb[:, hs]
                        acc = [pac.tile([128, 512], f32, tag="acc", bufs=4,
                                        name=f"acc{h}_{n}") for n in range(2)]
                        gi = [0]

                        def acc_branch(br_tile, ci):
                            diag = brp.tile([128, 128], f32r, tag="d", bufs=2,
                                            name=f"d{h}_{gi[0]}")
                            nc.vector.tensor_scalar_mul(diag, identity,
                                                        coeffs_bc[:, ci:ci + 1])
                            for n in range(2):
                                nc.tensor.matmul(acc[n], diag,
                                                 br_tile[:, n * 512:(n + 1) * 512],
                                                 start=(gi[0] == 0),
                                                 stop=(gi[0] == n_groups - 1))
                            gi[0] += 1

                        # --- nl_exp table phase ---
                        relu_br = brp.tile([128, HH], f32r, tag="relu",
                                           bufs=2, name=f"rl{h}")
                        nc.scalar.activation(relu_br, s_h, AF.Relu)
                        acc_branch(relu_br, 5)
                        # exp(min(s,0)); the -1 of expm1 is folded into the
                        # final subtraction of c_em below
                        mn = brp.tile([128, HH], f32, tag="sc1", bufs=2,
                                      name=f"mn{h}")
                        nc.vector.tensor_scalar_min(mn, s_h, 0.0)
                        em_br = brp.tile([128, HH], f32r, tag="b", bufs=2,
                                         name=f"em{h}")
                        nc.scalar.activation(em_br, mn, AF.Exp)
                        acc_branch(em_br, 6)
                        # mish = s * tanh(softplus(s)) computed without
                        # ln/tanh ACT passes (ACT is the tail bottleneck):
                        # tanh(softplus(s)) == 1 - 2/((1+e^s)^2 + 1), exact
                        # for s <= 20 and saturated to 1 in f32 above it; the
                        # DVE reciprocal is slow but overlaps the ACT stream.
                        mn20 = brp.tile([128, HH], f32, tag="sc2", bufs=2,
                                        name=f"m20{h}")
                        nc.vector.tensor_scalar_min(mn20, s_h, 20.0)
                        v_br = brp.tile([128, HH], f32, tag="sc1", bufs=2,
                                        name=f"v{h}")
                        nc.scalar.activation(v_br, mn20, AF.Exp)
                        vp1 = brp.tile([128, HH], f32, tag="sc2", bufs=2,
                                       name=f"vp{h}")
                        nc.vector.tensor_scalar_add(vp1, v_br, 1.0)
                        w_t = brp.tile([128, HH], f32, tag="sc1", bufs=2,
                                       name=f"w{h}")
                        nc.vector.tensor_tensor(w_t, vp1, vp1, ALU.mult)
                        wp1 = brp.tile([128, HH], f32, tag="sc2", bufs=2,
                                       name=f"wp{h}")
                        nc.vector.tensor_scalar_add(wp1, w_t, 1.0)
                        r_t = brp.tile([128, HH], f32, tag="sc1", bufs=2,
                                       name=f"r{h}")
                        nc.vector.reciprocal(r_t, wp1)
                        tsp = brp.tile([128, HH], f32, tag="sc2", bufs=2,
                                       name=f"t{h}")
                        nc.vector.tensor_scalar(tsp, r_t, -2.0, 1.0,
                                                ALU.mult, ALU.add)
                        mish_br = brp.tile([128, HH], f32r, tag="b", bufs=2,
                                           name=f"mi{h}")
                        nc.vector.tensor_tensor(mish_br, tsp, s_h, ALU.mult)
                        acc_branch(mish_br, 4)
                        # --- sigmoid table phase ---
                        sg_br = brp.tile([128, HH], f32r, tag="b", bufs=2,
                                         name=f"sg{h}")
                        nc.scalar.activation(sg_br, s_h, AF.Sigmoid)
                        acc_branch(sg_br, 0)
                        th_br = brp.tile([128, HH], f32r, tag="b", bufs=2,
                                         name=f"th{h}")
                        nc.scalar.activation(th_br, s_h, AF.Tanh)
                        acc_branch(th_br, 1)
                        # silu = s * sigmoid(s), on the vector engine
                        sl_br = brp.tile([128, HH], f32r, tag="b", bufs=2,
                                         name=f"sl{h}")
                        nc.vector.tensor_tensor(sl_br, sg_br.bitcast(f32), s_h,
                                                ALU.mult)
                        acc_branch(sl_br, 2)
                        # --- gelu table phase ---
                        gl_br = brp.tile([128, HH], f32r, tag="b", bufs=2,
                                         name=f"gl{h}")
                        nc.scalar.activation(gl_br, s_h, AF.Gelu)
                        acc_branch(gl_br, 3)
                        assert gi[0] == n_groups
                        for n in range(2):
                            sl = slice(h * HH + n * 512, h * HH + (n + 1) * 512)
                            nc.vector.tensor_scalar_sub(out_sb[:, sl], acc[n],
                                                        coeffs_bc[:, 6:7])
                        dma.dma_start(out_d[:, hs], out_sb[:, hs])
    nc.finalize()
    return nc


def _get_nc(key=None):
    if key is None:
        key = _LAST_KEY
    if key not in _CACHED_NC:
        _CACHED_NC[key] = _build_program(*key)
    return _CACHED_NC[key]


def _r12(a):
    """Round fp32 to the fp32r grid (11 explicit mantissa bits, RNE)."""
    u = np.ascontiguousarray(a).view(np.uint32)
    u = (u + np.uint32(0x7FF) + ((u >> np.uint32(12)) & np.uint32(1))) \
        & np.uint32(0xFFFFF000)
    return u.view(np.float32)


def kernel(**inputs):
    import os
    from concourse.bass_utils import run_bass_kernel_spmd

    f = lambda a: np.ascontiguousarray(np.asarray(a, dtype=np.float32))
    x = f(inputs["x"])
    gate_w = f(inputs["gate_w"])
    expert_w = f(inputs["expert_w"])
    expert_b = f(inputs["expert_b"])
    conn_w1 = f(inputs["conn_w1"])
    conn_b1 = f(inputs["conn_b1"])
    conn_w2 = f(inputs["conn_w2"])
    conn_b2 = f(inputs["conn_b2"])
    neuron_avg = f(inputs["neuron_avg"])
    neuron_mask = f(inputs["neuron_mask"])
    mem_read_w = f(inputs["mem_read_w"])
    mem_read_b = f(inputs["mem_read_b"])
    memory = f(inputs["memory"])
    act_w = f(inputs["act_w"]).reshape(-1)

    g = int(os.environ.get("MOE_G", "1"))
    EL = E // g
    NB = g

    # host prep: softmax blend weights -> 7 branch coefficients
    p = np.exp(act_w - act_w.max())
    p = p / p.sum()
    coef = np.array([[p[0], p[2], p[4], p[5], p[7],
                      p[1] + p[3] + p[6] * SELU_SCALE,
                      p[1] + p[6] * SELU_SCALE * SELU_ALPHA, 0.0]], np.float32)

    # host conn MLP (batch-independent) -> cmask folded into expert weights
    h1v = np.einsum('eh,ehk->ek', neuron_avg, conn_w1) + conn_b1
    h1v = np.maximum(h1v, 0.0, dtype=np.float32)
    cl = np.einsum('ek,ekh->eh', h1v, conn_w2) + conn_b2
    conn = (1.0 / (1.0 + np.exp(-cl))).astype(np.float32)
    cmask = conn * neuron_mask                                   # [E, H]

    # stage-1 live width: columns past the last nonzero mask column are
    # structurally zero in moe_out, so the program skips them entirely
    nz = np.nonzero(neuron_mask.any(axis=0))[0]
    h1 = int(nz[-1]) + 1 if nz.size else 512
    h1 = min(H, max(512, -(-h1 // 512) * 512))

    wp = (expert_w[:, :, :h1] * cmask[:, None, :h1]).astype(np.float16)
    bp = (expert_b[:, :h1] * cmask[:, :h1]).astype(np.float16)
    has_eb = bool(np.any(bp))
    has_mrb = bool(np.any(mem_read_b))

    xT = np.ascontiguousarray(x.T).astype(np.float16)            # [D, B]
    xT_blk = xT.reshape(8, 128, B).transpose(1, 0, 2)            # [128, 8, B]
    mrw_bf = mem_read_w[:h1].reshape(h1 // 128, 128, M).astype(np.float16)
    mem_bf = memory.reshape(16, 128, H).astype(np.float16)
    mrb = np.ascontiguousarray(mem_read_b.reshape(1, M))

    in_maps = []
    for c in range(NCORES):
        if g > 1:
            bg, eg = c >> 1, c & 1
        else:
            bg, eg = c, 0
        gwr = np.roll(gate_w, -eg * EL, axis=1).astype(np.float16)
        ew_c = wp[eg * EL:(eg + 1) * EL]         # [EL, D, h1]
        m = {
            "xT": np.ascontiguousarray(
                xT_blk[:, :, bg * NB * 128:(bg + 1) * NB * 128]),
            "gw": np.ascontiguousarray(gwr.reshape(8, 128, E).transpose(1, 0, 2)),
            "ew": np.ascontiguousarray(
                ew_c.reshape(EL, 8, 128, h1).transpose(0, 2, 1, 3)),
            "mrw": mrw_bf,
            "mem": mem_bf,
            "coef": coef,
        }
        if has_eb:
            m["eb"] = np.ascontiguousarray(
                bp[eg * EL:(eg + 1) * EL].reshape(EL, 1, h1))
        if has_mrb:
            m["mrb"] = mrb
        in_maps.append(m)

    key = (h1, g, has_eb, has_mrb)
    global _LAST_IN_MAPS, _LAST_KEY
    _LAST_IN_MAPS = in_maps
    _LAST_KEY = key
    nc = _get_nc(key)
    results = run_bass_kernel_spmd(nc, in_maps, list(range(NCORES))).results
    out = np.concatenate(
        [np.asarray(results[c]["out"], dtype=np.float32) for c in range(NCORES)],
        axis=0)
    return out
